# revision 40
# baseline (speedup 1.0000x reference)
"""Low-rank (CPD) 3D conv kernel for Trainium2, SPMD across 8 NeuronCores.

Math (per reference):
  y[r,h,w,d]  = sum_c U_c_in[c,r] * x[c,h,w,d]
  z           = conv_h(conv_w(conv_d(y)))   (separable 3-tap, per-rank taps)
  out[c,...]  = sum_r U_c_out[r,c] * z[r,...] + bias[c]

Distribution: data-parallel split of H (64) into 8 slabs of 8 planes; each
core reads its slab plus one halo plane on each side (zero at global edges)
and computes its output slab independently. No collectives.

Per-core pipeline (software-pipelined over planes, p = h + 3):
  - mm1 per INPUT plane (1x flops; the old kernel folded conv_h here at 3x):
    y[p] = U_c_in^T x[p], PSUM accumulated over 2 c-tiles, ACT-drained
    dense to bf16.
  - conv_h on DVE: 2 fused scalar_tensor_tensor passes per rank-tile using
    tap ratios U0/U1, U2/U1 (the U1 scale is folded into the mm2 weights),
    full-plane aligned -> 2x DVE mode.
  - conv_w on DVE: same 2-pass STT trick with +-1 w-line shifts (aligned),
    writing into a zero-padded z layout (66-wide d-lines, data in [0:64),
    pads stay zero) so mm2 can read d-shifted views safely.
  - conv_d is folded into mm2: out = sum_{k,rt} W_k[rt] @ z_shift(k) where
    W_k = U_kh[1]*U_kw[1]*U_kd[k] * U_c_out and z_shift(k) is a strided AP
    at element offset k-1 into the padded z lines (PE reads are
    alignment-insensitive; the padding zeros implement d-edge zero-pad).
  - mm2 drain on ACT with per-partition bias, bf16 output (host upcasts to
    f32), halving output DMA.
"""

import numpy as np
import ml_dtypes

BF16 = ml_dtypes.bfloat16

# Problem constants (hardcoded per contest contract)
C = 256   # input channels
R = 256   # rank
CO = 256  # output channels
S = 64    # spatial extent (cube)
NCORES = 8
HP = S // NCORES          # output planes per core (8)
HS = HP + 2               # slab planes incl. halo (10)
PLANE = S * S             # 4096 elements per (w,d) plane
ZLINE = S + 2             # padded d-line length (66)
ZPAD = 2 + ZLINE * S + 2  # padded z tile free dim (guards + 64 lines)

_cache = {}


def _build_program(hp=HP):
    """Build and compile the per-core Bass program (identical on all cores)."""
    import concourse.bass as bass
    import concourse.mybir as mybir
    import concourse.tile as tile
    from concourse import bacc

    HS_, HP_ = hp + 2, hp

    fp32 = mybir.dt.float32
    bf16 = mybir.dt.bfloat16
    mult = mybir.AluOpType.mult
    add = mybir.AluOpType.add
    ident = mybir.ActivationFunctionType.Identity

    nc = bacc.Bacc("TRN2", target_bir_lowering=False, debug=False,
                   num_devices=NCORES)

    # DRAM tensors (names are the in_map keys)
    x_d = nc.dram_tensor("xs", [2, 128, HS_, PLANE], bf16, kind="ExternalInput").ap()
    w1_d = nc.dram_tensor("w1", [2, 2, 128, 128], bf16, kind="ExternalInput").ap()
    w2_d = nc.dram_tensor("w2", [3, 2, 2, 128, 128], bf16, kind="ExternalInput").ap()
    rh_d = nc.dram_tensor("rh", [2, 128, 2], fp32, kind="ExternalInput").ap()
    rw_d = nc.dram_tensor("rw", [2, 128, 2], fp32, kind="ExternalInput").ap()
    bias_d = nc.dram_tensor("bias_t", [2, 128, 1], fp32, kind="ExternalInput").ap()
    out_d = nc.dram_tensor("out", [2, 128, HP_, PLANE], bf16, kind="ExternalOutput").ap()

    with tile.TileContext(nc) as tc:
        consts = tc.alloc_tile_pool(name="consts", bufs=1)
        xpool = tc.alloc_tile_pool(name="x", bufs=4)
        ypool = tc.alloc_tile_pool(name="y", bufs=8)
        tpool = tc.alloc_tile_pool(name="tmp", bufs=5)
        gpool = tc.alloc_tile_pool(name="gtmp", bufs=3)
        zpool = tc.alloc_tile_pool(name="z", bufs=1)
        opool = tc.alloc_tile_pool(name="osb", bufs=2)
        ps1 = tc.alloc_tile_pool(name="ps1", bufs=2, space="PSUM")
        ps2 = tc.alloc_tile_pool(name="ps2", bufs=2, space="PSUM")

        # ---- x plane streaming (x(0) DMA first: it gates mm1(0)) ----
        xt = {}

        def get_x(p, ct):
            if (p, ct) not in xt:
                t = xpool.tile([128, PLANE], bf16, name="xplane", tag="xplane")
                nc.sync.dma_start(out=t[:, 0:PLANE // 2],
                                  in_=x_d[ct, :, p, 0:PLANE // 2])
                nc.sync.dma_start(out=t[:, PLANE // 2:],
                                  in_=x_d[ct, :, p, PLANE // 2:])
                xt[(p, ct)] = t
            return xt[(p, ct)]

        # ---- constants (w1 first: it gates the very first matmul) ----
        w1 = [[consts.tile([128, 128], bf16, name=f"w1_{ct}{rt}", tag=f"w1_{ct}{rt}")
               for rt in range(2)] for ct in range(2)]
        for ct in range(2):
            for rt in range(2):
                nc.sync.dma_start(out=w1[ct][rt], in_=w1_d[ct, rt])
        for ct in range(2):
            get_x(0, ct)
        w2 = [[[consts.tile([128, 128], bf16, name=f"w2_{k}{rt}{co}", tag=f"w2_{k}{rt}{co}")
                for co in range(2)] for rt in range(2)] for k in range(3)]
        for k in range(3):
            for rt in range(2):
                for co in range(2):
                    nc.sync.dma_start(out=w2[k][rt][co], in_=w2_d[k, rt, co])
        rh = [consts.tile([128, 2], fp32, name=f"rh{rt}", tag=f"rh{rt}") for rt in range(2)]
        rw = [consts.tile([128, 2], fp32, name=f"rw{rt}", tag=f"rw{rt}") for rt in range(2)]
        bia = [consts.tile([128, 1], fp32, name=f"bias{co}", tag=f"bias{co}") for co in range(2)]
        for rt in range(2):
            nc.sync.dma_start(out=rh[rt], in_=rh_d[rt])
            nc.sync.dma_start(out=rw[rt], in_=rw_d[rt])
        for co in range(2):
            nc.sync.dma_start(out=bia[co], in_=bias_d[co])

        # ---- persistent padded z tiles (pads memset once, stay zero) ----
        zt = {}
        for slot in range(2):
            for rt in range(2):
                t = zpool.tile([128, ZPAD], bf16, name=f"zt{slot}{rt}",
                               tag=f"zt{slot}{rt}")
                # only guards + per-line pad slots need zeroing
                nc.vector.memset(t[:, 0:2], 0.0)
                nc.vector.memset(t[:, ZPAD - 2:ZPAD], 0.0)
                nc.vector.memset(
                    t[:, 2:2 + ZLINE * S].rearrange(
                        "p (w e) -> p w e", e=ZLINE)[:, :, S:ZLINE], 0.0)
                zt[(slot, rt)] = t

        def zlines(slot, rt):
            # [128, 64 lines, 64 data] view of the padded z tile
            return zt[(slot, rt)][:, 2:2 + ZLINE * S].rearrange(
                "p (w e) -> p w e", e=ZLINE)[:, :, 0:S]

        def zrhs(slot, rt, q, k):
            # mm2 moving operand: 8 w-lines x 64 cols at d-offset (k-1)
            b = 2 + ZLINE * (8 * q) + (k - 1)
            return zt[(slot, rt)][:, b:b + 8 * ZLINE].rearrange(
                "p (w e) -> p w e", e=ZLINE)[:, :, 0:S]

        # ---- PE warm-up: dummy MMs during the prologue DMA wait ----
        wsc = consts.tile([128, 64], bf16, name="warm", tag="warm")
        nc.vector.memset(wsc, 0.0)
        wps = ps2.tile([128, 1024], fp32, name="wps", tag="ps2")
        for i in range(48):
            nc.tensor.matmul(wps[0:64, i % 8 * 64:(i % 8 + 1) * 64], wsc, wsc,
                             start=(i < 8), stop=(i >= 40),
                             skip_group_check=True)
        wsb = consts.tile([64, 64], bf16, name="wsb", tag="wsb")
        nc.scalar.copy(wsb, wps[0:64, 0:64])

        yt = {}  # (p%4, rt) -> dense bf16 y tile

        def mm1(p):
            for rt in (1, 0):  # rt1 first: it gates each phase's DVE/GpSimd chain head
                if (p % 4, rt) not in yt:
                    yt[(p % 4, rt)] = ypool.tile([128, PLANE], bf16,
                                                 name="yplane", tag="yplane")
                ysb = yt[(p % 4, rt)]
                for qq in range(4):
                    pt = ps1.tile([128, 1024], fp32, name="pt", tag="ps1")
                    for ct in range(2):
                        for c2 in range(2):
                            q = qq * 2 + c2
                            nc.tensor.matmul(
                                pt[:, c2 * 512:(c2 + 1) * 512],
                                w1[ct][rt],
                                get_x(p, ct)[:, q * 512:(q + 1) * 512],
                                start=(ct == 0),
                                stop=(ct == 1),
                                skip_group_check=True,
                            )
                    nc.scalar.copy(ysb[:, qq * 1024:(qq + 1) * 1024], pt)

        heads = {}
        CB = 33 * S          # asym half boundary (2112 cols = lines 0..32)
        SY = 32 * S          # sym half boundary (2048 cols)

        def conv_head(h):
            """Chain heads: DVE scale for rt1, then its conv_h add on GpSimd
            in two chunks (so the rt1 half-0 chain is not gated by a full
            GpSimd plane), then DVE scale for rt0."""
            y = {(i, rt): yt[((h + i) % 4, rt)]
                 for i in range(2) for rt in range(2)}
            sy0_1 = gpool.tile([128, PLANE], bf16, name="sy0_1", tag="gtmp")
            nc.vector.tensor_scalar_mul(sy0_1, y[(0, 1)], rh[1][:, 0:1])
            th1 = gpool.tile([128, PLANE], bf16, name="th1", tag="gtmp")
            for a0, a1 in ((0, CB), (CB, PLANE)):
                nc.gpsimd.tensor_tensor(
                    th1[:, a0:a1], sy0_1[:, a0:a1], y[(1, 1)][:, a0:a1], add)
            sy0 = gpool.tile([128, PLANE], bf16, name="sy0", tag="gtmp")
            nc.vector.tensor_scalar_mul(sy0, y[(0, 0)], rh[0][:, 0:1])
            heads[h] = (th1, sy0)

        def conv(h):
            """conv_h + conv_w for out-plane h -> padded z[(h%2, rt)].

            Half-major order: both rank-tiles finish w-lines 0..31 of z
            before lines 32..63, so next phase's mm2 (issued qq-major) can
            start on the first z half early. Tiles alias across stages
            (th+sa, sy2+sa2, a+t2 in-place) to keep the live set at 5; all
            clobbers happen after the last read of the previous tenant.
            """
            slot, (th1, sy0) = h % 2, heads.pop(h)
            y = {(i, rt): yt[((h + i) % 4, rt)]
                 for i in range(3) for rt in range(2)}
            tA = {1: th1}  # rt1 th/sa tile is the gpool th1
            tA[0] = tpool.tile([128, PLANE], bf16, name="thsa", tag="tmp")
            tB = {rt: tpool.tile([128, PLANE], bf16, name="s2s2", tag="tmp")
                  for rt in range(2)}
            tC = {rt: tpool.tile([128, PLANE], bf16, name="at2", tag="tmp")
                  for rt in range(2)}
            for hf in range(2):
                A = slice(0, CB) if hf == 0 else slice(CB, PLANE)   # asym
                Y = slice(0, SY) if hf == 0 else slice(SY, PLANE)   # sym
                for rt in range(2):
                    th, b, c = tA[rt], tB[rt], tC[rt]
                    if rt == 0:
                        nc.vector.tensor_tensor(
                            th[:, A], sy0[:, A], y[(1, 0)][:, A], add)
                    nc.vector.tensor_scalar_mul(
                        b[:, A], y[(2, rt)][:, A], rh[rt][:, 1:2])
                    nc.vector.tensor_tensor(c[:, A], b[:, A], th[:, A], add)
                    # conv_w: sa (sym) into th's tile; sa2 (asym) into sy2's
                    nc.vector.tensor_scalar_mul(th[:, Y], c[:, Y],
                                                rw[rt][:, 0:1])
                    nc.vector.tensor_scalar_mul(b[:, A], c[:, A],
                                                rw[rt][:, 1:2])
                    sav = th.rearrange("p (w q) -> p w q", q=S)
                    av = c.rearrange("p (w q) -> p w q", q=S)
                    sa2v = b.rearrange("p (w q) -> p w q", q=S)
                    zv = zlines(slot, rt)
                    if hf == 0:
                        # t2[w]=r0w*a[w-1]+a[w] in-place on a (w=1..31)
                        nc.vector.tensor_tensor(
                            av[:, 1:32, :], sav[:, 0:31, :], av[:, 1:32, :],
                            add)
                        nc.vector.tensor_tensor(
                            zv[:, 0:32, :], sa2v[:, 1:33, :], av[:, 0:32, :],
                            add)
                    else:
                        nc.vector.tensor_tensor(
                            av[:, 32:, :], sav[:, 31:63, :], av[:, 32:, :],
                            add)
                        nc.vector.tensor_tensor(
                            zv[:, 32:63, :], sa2v[:, 33:, :], av[:, 32:63, :],
                            add)
                        nc.vector.tensor_copy(zv[:, 63, :], av[:, 63, :])

        def mm2(h):
            slot = h % 2
            for qq in range(4):
                for co in range(2):
                    pt = ps2.tile([128, 1024], fp32, name="pt2", tag="ps2")
                    n = 0
                    for rt in range(2):
                        for k in range(3):
                            for c2 in range(2):
                                q = qq * 2 + c2
                                nc.tensor.matmul(
                                    pt[:, c2 * 512:(c2 + 1) * 512],
                                    w2[k][rt][co],
                                    zrhs(slot, rt, q, k),
                                    start=(n < 2),
                                    stop=(n >= 10),
                                    skip_group_check=True,
                                )
                                n += 1
                    osb = opool.tile([128, 1024], bf16, name="osb", tag="osb")
                    nc.scalar.activation(osb, pt, ident, bias=bia[co][:, 0:1])
                    nc.sync.dma_start(
                        out=out_d[co, :, h, qq * 1024:(qq + 1) * 1024],
                        in_=osb)

        # --- software pipeline: phase h issues mm1(h+4), conv(h+1), mm2(h)
        # so PE's mm2 only depends on the PREVIOUS phase's DVE output.
        for p in range(4):
            for ct in range(2):
                get_x(p, ct)
        mm1(0)
        mm1(1)
        mm1(2)
        conv_head(0)
        conv(0)
        mm1(3)
        for ct in range(2):
            get_x(4, ct)

        for h in range(HP_):
            p = h + 4
            if h + 1 < HP_:
                conv_head(h + 1)
            if p + 1 < HS_:
                for ct in range(2):
                    get_x(p + 1, ct)
            if p < HS_:
                mm1(p)
            if h + 1 < HP_:
                conv(h + 1)
            mm2(h)

        for pool in (ps2, ps1, opool, zpool, gpool, tpool, ypool, xpool, consts):
            pool.release()

    nc.compile()
    return nc


def _host_prep(x, U_kh, U_kw, U_kd, U_c_in, U_c_out, bias):
    """Build per-core input maps (numpy only)."""
    x = np.asarray(x)
    U_kh = np.asarray(U_kh, np.float32)
    U_kw = np.asarray(U_kw, np.float32)
    U_kd = np.asarray(U_kd, np.float32)
    U_c_in = np.asarray(U_c_in, np.float32)
    U_c_out = np.asarray(U_c_out, np.float32)
    bias = np.asarray(bias, np.float32)

    xb = np.ascontiguousarray(x[0]).astype(BF16)          # [C, S, S, S]
    xb = xb.reshape(C, S, PLANE)

    # mm1 weights: U_c_in blocks [ct, rt, 128, 128]
    w1 = np.ascontiguousarray(
        U_c_in.astype(BF16).reshape(2, 128, 2, 128).transpose(0, 2, 1, 3))

    # mm2 weights with conv_d taps + U1h*U1w rescale folded in:
    # W_k[r, co] = U_kh[1,r]*U_kw[1,r]*U_kd[k,r]*U_c_out[r,co]
    w2 = np.empty((3, 2, 2, 128, 128), BF16)
    scale_r = U_kh[1] * U_kw[1]                            # [R]
    for k in range(3):
        wk = (scale_r * U_kd[k])[:, None] * U_c_out        # [R, CO]
        w2[k] = wk.astype(BF16).reshape(2, 128, 2, 128).transpose(0, 2, 1, 3)

    # tap ratios for the STT conv passes
    rh = np.stack([U_kh[0] / U_kh[1], U_kh[2] / U_kh[1]], axis=1)  # [R, 2]
    rw = np.stack([U_kw[0] / U_kw[1], U_kw[2] / U_kw[1]], axis=1)
    rh = np.ascontiguousarray(rh.reshape(2, 128, 2).astype(np.float32))
    rw = np.ascontiguousarray(rw.reshape(2, 128, 2).astype(np.float32))
    bias_t = np.ascontiguousarray(bias.reshape(2, 128, 1))

    in_maps = []
    for c in range(NCORES):
        slab = np.zeros((C, HS, PLANE), BF16)
        lo, hi = c * HP - 1, c * HP + HP + 1
        s0, s1 = max(lo, 0), min(hi, S)
        slab[:, s0 - lo:HS - (hi - s1)] = xb[:, s0:s1]
        slab = np.ascontiguousarray(slab.reshape(2, 128, HS, PLANE))
        in_maps.append({
            "xs": slab, "w1": w1, "w2": w2, "rh": rh, "rw": rw,
            "bias_t": bias_t,
        })
    return in_maps


def kernel(x, U_kh, U_kw, U_kd, U_c_in, U_c_out, bias, _trace=False):
    from concourse.bass_utils import run_bass_kernel_spmd

    if "nc" not in _cache:
        _cache["nc"] = _build_program()
    nc = _cache["nc"]

    in_maps = _host_prep(x, U_kh, U_kw, U_kd, U_c_in, U_c_out, bias)
    res = run_bass_kernel_spmd(nc, in_maps, core_ids=list(range(NCORES)),
                               trace=_trace)
    _cache["last_result"] = res

    out = np.empty((1, CO, S, S, S), np.float32)
    for c in range(NCORES):
        o = res.results[c]["out"]                        # [2, 128, HP, PLANE] bf16
        out[0, :, c * HP:(c + 1) * HP] = o.astype(np.float32).reshape(CO, HP, S, S)
    return out


# revision 42
# speedup vs baseline: 1.1954x; 1.1954x over previous
"""Low-rank (CPD) 3D conv kernel for Trainium2, SPMD across 8 NeuronCores.

Math (per reference):
  y[r,h,w,d]  = sum_c U_c_in[c,r] * x[c,h,w,d]
  z           = conv_h(conv_w(conv_d(y)))   (separable 3-tap, per-rank taps)
  out[c,...]  = sum_r U_c_out[r,c] * z[r,...] + bias[c]

Distribution: data-parallel split of H (64) into 8 slabs of 8 planes; each
core reads its slab plus one halo plane on each side (zero at global edges)
and computes its output slab independently. No collectives.

Per-core pipeline (software-pipelined over planes, p = h + 3):
  - mm1 per INPUT plane (1x flops; the old kernel folded conv_h here at 3x):
    y[p] = U_c_in^T x[p], PSUM accumulated over 2 c-tiles, ACT-drained
    dense to bf16.
  - conv_h on DVE: 2 fused scalar_tensor_tensor passes per rank-tile using
    tap ratios U0/U1, U2/U1 (the U1 scale is folded into the mm2 weights),
    full-plane aligned -> 2x DVE mode.
  - conv_w on DVE: same 2-pass STT trick with +-1 w-line shifts (aligned),
    writing into a zero-padded z layout (66-wide d-lines, data in [0:64),
    pads stay zero) so mm2 can read d-shifted views safely.
  - conv_d is folded into mm2: out = sum_{k,rt} W_k[rt] @ z_shift(k) where
    W_k = U_kh[1]*U_kw[1]*U_kd[k] * U_c_out and z_shift(k) is a strided AP
    at element offset k-1 into the padded z lines (PE reads are
    alignment-insensitive; the padding zeros implement d-edge zero-pad).
  - mm2 drain on ACT with per-partition bias, bf16 output (host upcasts to
    f32), halving output DMA.
"""

import numpy as np
import ml_dtypes

BF16 = ml_dtypes.bfloat16

# Problem constants (hardcoded per contest contract)
C = 256   # input channels
R = 256   # rank
CO = 256  # output channels
S = 64    # spatial extent (cube)
NCORES = 8
HP = S // NCORES          # output planes per core (8)
HS = HP + 2               # slab planes incl. halo (10)
PLANE = S * S             # 4096 elements per (w,d) plane
ZLINE = S + 2             # padded d-line length (66)
ZPAD = 2 + ZLINE * S + 2  # padded z tile free dim (guards + 64 lines)

_cache = {}


def _build_program(hp=HP):
    """Build and compile the per-core Bass program (identical on all cores)."""
    import concourse.bass as bass
    import concourse.mybir as mybir
    import concourse.tile as tile
    from concourse import bacc

    HS_, HP_ = hp + 2, hp

    fp32 = mybir.dt.float32
    bf16 = mybir.dt.bfloat16
    mult = mybir.AluOpType.mult
    add = mybir.AluOpType.add
    ident = mybir.ActivationFunctionType.Identity

    nc = bacc.Bacc("TRN2", target_bir_lowering=False, debug=False,
                   num_devices=NCORES)

    # DRAM tensors (names are the in_map keys)
    x_d = nc.dram_tensor("xs", [2, 128, HS_, PLANE], bf16, kind="ExternalInput").ap()
    w1_d = nc.dram_tensor("w1", [2, 2, 128, 128], bf16, kind="ExternalInput").ap()
    w2_d = nc.dram_tensor("w2", [3, 2, 2, 128, 128], bf16, kind="ExternalInput").ap()
    rh_d = nc.dram_tensor("rh", [2, 128, 2], fp32, kind="ExternalInput").ap()
    rw_d = nc.dram_tensor("rw", [2, 128, 2], fp32, kind="ExternalInput").ap()
    bias_d = nc.dram_tensor("bias_t", [2, 128, 1], fp32, kind="ExternalInput").ap()
    out_d = nc.dram_tensor("out", [2, 128, HP_, PLANE], bf16, kind="ExternalOutput").ap()

    with tile.TileContext(nc) as tc:
        consts = tc.alloc_tile_pool(name="consts", bufs=1)
        xpool = tc.alloc_tile_pool(name="x", bufs=4)
        ypool = tc.alloc_tile_pool(name="y", bufs=8)
        tpool = tc.alloc_tile_pool(name="tmp", bufs=3)
        gpool = tc.alloc_tile_pool(name="gtmp", bufs=3)
        zpool = tc.alloc_tile_pool(name="z", bufs=1)
        opool = tc.alloc_tile_pool(name="osb", bufs=2)
        ps1 = tc.alloc_tile_pool(name="ps1", bufs=2, space="PSUM")
        ps2 = tc.alloc_tile_pool(name="ps2", bufs=2, space="PSUM")

        # ---- x plane streaming (x(0) DMA first: it gates mm1(0)) ----
        xt = {}

        def get_x(p, ct):
            if (p, ct) not in xt:
                t = xpool.tile([128, PLANE], bf16, name="xplane", tag="xplane")
                nc.sync.dma_start(out=t[:, 0:PLANE // 2],
                                  in_=x_d[ct, :, p, 0:PLANE // 2])
                nc.sync.dma_start(out=t[:, PLANE // 2:],
                                  in_=x_d[ct, :, p, PLANE // 2:])
                xt[(p, ct)] = t
            return xt[(p, ct)]

        # ---- constants (w1 first: it gates the very first matmul) ----
        w1 = [[consts.tile([128, 128], bf16, name=f"w1_{ct}{rt}", tag=f"w1_{ct}{rt}")
               for rt in range(2)] for ct in range(2)]
        for ct in range(2):
            for rt in range(2):
                nc.sync.dma_start(out=w1[ct][rt], in_=w1_d[ct, rt])
        for ct in range(2):
            get_x(0, ct)
        w2 = [[[consts.tile([128, 128], bf16, name=f"w2_{k}{rt}{co}", tag=f"w2_{k}{rt}{co}")
                for co in range(2)] for rt in range(2)] for k in range(3)]
        for k in range(3):
            for rt in range(2):
                for co in range(2):
                    nc.sync.dma_start(out=w2[k][rt][co], in_=w2_d[k, rt, co])
        rh = [consts.tile([128, 2], fp32, name=f"rh{rt}", tag=f"rh{rt}") for rt in range(2)]
        rw = [consts.tile([128, 2], fp32, name=f"rw{rt}", tag=f"rw{rt}") for rt in range(2)]
        bia = [consts.tile([128, 1], fp32, name=f"bias{co}", tag=f"bias{co}") for co in range(2)]
        for rt in range(2):
            nc.sync.dma_start(out=rh[rt], in_=rh_d[rt])
            nc.sync.dma_start(out=rw[rt], in_=rw_d[rt])
        for co in range(2):
            nc.sync.dma_start(out=bia[co], in_=bias_d[co])

        # ---- persistent padded z tiles (pads memset once, stay zero) ----
        zt = {}
        for slot in range(2):
            for rt in range(2):
                t = zpool.tile([128, ZPAD], bf16, name=f"zt{slot}{rt}",
                               tag=f"zt{slot}{rt}")
                # only guards + per-line pad slots need zeroing
                nc.vector.memset(t[:, 0:2], 0.0)
                nc.vector.memset(t[:, ZPAD - 2:ZPAD], 0.0)
                nc.vector.memset(
                    t[:, 2:2 + ZLINE * S].rearrange(
                        "p (w e) -> p w e", e=ZLINE)[:, :, S:ZLINE], 0.0)
                zt[(slot, rt)] = t

        # persistent sa2 tiles: 65 lines, line 64 stays zero so the z-h1
        # TT can fold the w=63 boundary (r2w*a[64]=0) instead of a copy
        tBt = {}
        for rt in range(2):
            t = zpool.tile([128, PLANE + S], bf16, name=f"tB{rt}",
                           tag=f"tB{rt}")
            nc.vector.memset(t[:, PLANE:], 0.0)
            tBt[rt] = t

        def zlines(slot, rt):
            # [128, 64 lines, 64 data] view of the padded z tile
            return zt[(slot, rt)][:, 2:2 + ZLINE * S].rearrange(
                "p (w e) -> p w e", e=ZLINE)[:, :, 0:S]

        def zrhs(slot, rt, q, k):
            # mm2 moving operand: 8 w-lines x 64 cols at d-offset (k-1)
            b = 2 + ZLINE * (8 * q) + (k - 1)
            return zt[(slot, rt)][:, b:b + 8 * ZLINE].rearrange(
                "p (w e) -> p w e", e=ZLINE)[:, :, 0:S]

        # ---- PE warm-up: dummy MMs during the prologue DMA wait ----
        wsc = consts.tile([128, 64], bf16, name="warm", tag="warm")
        nc.vector.memset(wsc, 0.0)
        wps = ps2.tile([128, 1024], fp32, name="wps", tag="ps2")
        for i in range(24):
            nc.tensor.matmul(wps[0:64, i % 8 * 64:(i % 8 + 1) * 64], wsc, wsc,
                             start=(i < 8), stop=(i >= 16),
                             skip_group_check=True)
        wsb = consts.tile([64, 64], bf16, name="wsb", tag="wsb")
        nc.scalar.copy(wsb, wps[0:64, 0:64])

        yt = {}  # (p%4, rt) -> dense bf16 y tile

        def mm1(p):
            for rt in (1, 0):  # rt1 first: it gates each phase's DVE/GpSimd chain head
                if (p % 4, rt) not in yt:
                    yt[(p % 4, rt)] = ypool.tile([128, PLANE], bf16,
                                                 name="yplane", tag="yplane")
                ysb = yt[(p % 4, rt)]
                for qq in range(4):
                    pt = ps1.tile([128, 1024], fp32, name="pt", tag="ps1")
                    for ct in range(2):
                        for c2 in range(2):
                            q = qq * 2 + c2
                            nc.tensor.matmul(
                                pt[:, c2 * 512:(c2 + 1) * 512],
                                w1[ct][rt],
                                get_x(p, ct)[:, q * 512:(q + 1) * 512],
                                start=(ct == 0),
                                stop=(ct == 1),
                                skip_group_check=True,
                            )
                    nc.scalar.copy(ysb[:, qq * 1024:(qq + 1) * 1024], pt)

        heads = {}
        CB = 33 * S          # asym half boundary (2112 cols = lines 0..32)
        SY = 32 * S          # sym half boundary (2048 cols)

        def conv_head(h):
            """Chain heads: DVE scale for rt1, then its conv_h add on GpSimd
            in two chunks (so the rt1 half-0 chain is not gated by a full
            GpSimd plane), then DVE scale for rt0."""
            y = {(i, rt): yt[((h + i) % 4, rt)]
                 for i in range(2) for rt in range(2)}
            sy0_1 = gpool.tile([128, PLANE], bf16, name="sy0_1", tag="gtmp")
            nc.vector.tensor_scalar_mul(sy0_1, y[(0, 1)], rh[1][:, 0:1])
            th1 = gpool.tile([128, PLANE], bf16, name="th1", tag="gtmp")
            for a0, a1 in ((0, CB), (CB, PLANE)):
                nc.gpsimd.tensor_tensor(
                    th1[:, a0:a1], sy0_1[:, a0:a1], y[(1, 1)][:, a0:a1], add)
            sy0 = gpool.tile([128, PLANE], bf16, name="sy0", tag="gtmp")
            nc.vector.tensor_scalar_mul(sy0, y[(0, 0)], rh[0][:, 0:1])
            heads[h] = (th1, sy0)

        def conv(h):
            """conv_h + conv_w for out-plane h -> padded z[(h%2, rt)].

            Half-major order: both rank-tiles finish w-lines 0..31 of z
            before lines 32..63, so next phase's mm2 (issued qq-major) can
            start on the first z half early. Tiles alias across stages
            (th+sa, sy2+sa2, a+t2 in-place) to keep the live set at 5; all
            clobbers happen after the last read of the previous tenant.
            """
            slot, (th1, sy0) = h % 2, heads.pop(h)
            y = {(i, rt): yt[((h + i) % 4, rt)]
                 for i in range(3) for rt in range(2)}
            tA = {1: th1}  # rt1 th/sa tile is the gpool th1
            tA[0] = tpool.tile([128, PLANE], bf16, name="thsa", tag="tmp")
            tB = tBt
            tC = {rt: tpool.tile([128, PLANE], bf16, name="at2", tag="tmp")
                  for rt in range(2)}
            for hf in range(2):
                A = slice(0, CB) if hf == 0 else slice(CB, PLANE)   # asym
                Y = slice(0, SY) if hf == 0 else slice(SY, PLANE)   # sym
                for rt in range(2):
                    th, b, c = tA[rt], tB[rt], tC[rt]
                    if rt == 0:
                        nc.vector.tensor_tensor(
                            th[:, A], sy0[:, A], y[(1, 0)][:, A], add)
                    nc.vector.tensor_scalar_mul(
                        b[:, A], y[(2, rt)][:, A], rh[rt][:, 1:2])
                    nc.vector.tensor_tensor(c[:, A], b[:, A], th[:, A], add)
                    # conv_w: sa (sym) into th's tile; sa2 (asym) into sy2's
                    nc.vector.tensor_scalar_mul(th[:, Y], c[:, Y],
                                                rw[rt][:, 0:1])
                    nc.vector.tensor_scalar_mul(b[:, A], c[:, A],
                                                rw[rt][:, 1:2])
                    sav = th.rearrange("p (w q) -> p w q", q=S)
                    av = c.rearrange("p (w q) -> p w q", q=S)
                    sa2v = b.rearrange("p (w q) -> p w q", q=S)  # 65 lines
                    zv = zlines(slot, rt)
                    if hf == 0:
                        # t2[w]=r0w*a[w-1]+a[w] in-place on a (w=1..31)
                        nc.vector.tensor_tensor(
                            av[:, 1:32, :], sav[:, 0:31, :], av[:, 1:32, :],
                            add)
                        nc.vector.tensor_tensor(
                            zv[:, 0:32, :], sa2v[:, 1:33, :], av[:, 0:32, :],
                            add)
                    else:
                        nc.vector.tensor_tensor(
                            av[:, 32:, :], sav[:, 31:63, :], av[:, 32:, :],
                            add)
                        nc.vector.tensor_tensor(
                            zv[:, 32:64, :], sa2v[:, 33:65, :],
                            av[:, 32:64, :], add)

        def mm2(h):
            slot = h % 2
            for qq in range(4):
                for co in range(2):
                    pt = ps2.tile([128, 1024], fp32, name="pt2", tag="ps2")
                    n = 0
                    for rt in range(2):
                        for k in range(3):
                            for c2 in range(2):
                                q = qq * 2 + c2
                                nc.tensor.matmul(
                                    pt[:, c2 * 512:(c2 + 1) * 512],
                                    w2[k][rt][co],
                                    zrhs(slot, rt, q, k),
                                    start=(n < 2),
                                    stop=(n >= 10),
                                    skip_group_check=True,
                                )
                                n += 1
                    osb = opool.tile([128, 1024], bf16, name="osb", tag="osb")
                    nc.scalar.activation(osb, pt, ident, bias=bia[co][:, 0:1])
                    nc.sync.dma_start(
                        out=out_d[co, :, h, qq * 1024:(qq + 1) * 1024],
                        in_=osb)

        # --- software pipeline: phase h issues mm1(h+4), conv(h+1), mm2(h)
        # so PE's mm2 only depends on the PREVIOUS phase's DVE output.
        for p in range(4):
            for ct in range(2):
                get_x(p, ct)
        mm1(0)
        mm1(1)
        mm1(2)
        conv_head(0)
        conv(0)
        mm1(3)
        for ct in range(2):
            get_x(4, ct)

        for h in range(HP_):
            p = h + 4
            if h + 1 < HP_:
                conv_head(h + 1)
            if p + 1 < HS_:
                for ct in range(2):
                    get_x(p + 1, ct)
            if p < HS_:
                mm1(p)
            if h + 1 < HP_:
                conv(h + 1)
            mm2(h)

        for pool in (ps2, ps1, opool, zpool, gpool, tpool, ypool, xpool, consts):
            pool.release()

    nc.compile()
    return nc


def _host_prep(x, U_kh, U_kw, U_kd, U_c_in, U_c_out, bias):
    """Build per-core input maps (numpy only)."""
    x = np.asarray(x)
    U_kh = np.asarray(U_kh, np.float32)
    U_kw = np.asarray(U_kw, np.float32)
    U_kd = np.asarray(U_kd, np.float32)
    U_c_in = np.asarray(U_c_in, np.float32)
    U_c_out = np.asarray(U_c_out, np.float32)
    bias = np.asarray(bias, np.float32)

    xb = np.ascontiguousarray(x[0]).astype(BF16)          # [C, S, S, S]
    xb = xb.reshape(C, S, PLANE)

    # mm1 weights: U_c_in blocks [ct, rt, 128, 128]
    w1 = np.ascontiguousarray(
        U_c_in.astype(BF16).reshape(2, 128, 2, 128).transpose(0, 2, 1, 3))

    # mm2 weights with conv_d taps + U1h*U1w rescale folded in:
    # W_k[r, co] = U_kh[1,r]*U_kw[1,r]*U_kd[k,r]*U_c_out[r,co]
    w2 = np.empty((3, 2, 2, 128, 128), BF16)
    scale_r = U_kh[1] * U_kw[1]                            # [R]
    for k in range(3):
        wk = (scale_r * U_kd[k])[:, None] * U_c_out        # [R, CO]
        w2[k] = wk.astype(BF16).reshape(2, 128, 2, 128).transpose(0, 2, 1, 3)

    # tap ratios for the STT conv passes
    rh = np.stack([U_kh[0] / U_kh[1], U_kh[2] / U_kh[1]], axis=1)  # [R, 2]
    rw = np.stack([U_kw[0] / U_kw[1], U_kw[2] / U_kw[1]], axis=1)
    rh = np.ascontiguousarray(rh.reshape(2, 128, 2).astype(np.float32))
    rw = np.ascontiguousarray(rw.reshape(2, 128, 2).astype(np.float32))
    bias_t = np.ascontiguousarray(bias.reshape(2, 128, 1))

    in_maps = []
    for c in range(NCORES):
        slab = np.zeros((C, HS, PLANE), BF16)
        lo, hi = c * HP - 1, c * HP + HP + 1
        s0, s1 = max(lo, 0), min(hi, S)
        slab[:, s0 - lo:HS - (hi - s1)] = xb[:, s0:s1]
        slab = np.ascontiguousarray(slab.reshape(2, 128, HS, PLANE))
        in_maps.append({
            "xs": slab, "w1": w1, "w2": w2, "rh": rh, "rw": rw,
            "bias_t": bias_t,
        })
    return in_maps


def kernel(x, U_kh, U_kw, U_kd, U_c_in, U_c_out, bias, _trace=False):
    from concourse.bass_utils import run_bass_kernel_spmd

    if "nc" not in _cache:
        _cache["nc"] = _build_program()
    nc = _cache["nc"]

    in_maps = _host_prep(x, U_kh, U_kw, U_kd, U_c_in, U_c_out, bias)
    res = run_bass_kernel_spmd(nc, in_maps, core_ids=list(range(NCORES)),
                               trace=_trace)
    _cache["last_result"] = res

    out = np.empty((1, CO, S, S, S), np.float32)
    for c in range(NCORES):
        o = res.results[c]["out"]                        # [2, 128, HP, PLANE] bf16
        out[0, :, c * HP:(c + 1) * HP] = o.astype(np.float32).reshape(CO, HP, S, S)
    return out
